# revision 7
# baseline (speedup 1.0000x reference)
"""Trainium2 Bass kernel for BGEM3 sparse-embedding head (segment_reduce).

Computes, for inputs hidden_state [B,S,H], input_ids [B,S], W_sparse [1,H],
b_sparse [1]:
    tw = relu(hidden_state @ W_sparse[0] + b_sparse[0])          # [B,S]
    out = zeros([B,V]); out.at[b, ids].max(tw)  (jax scatter-max, which on
    this stack sums duplicate indices); out[:, 0:4] = 0
Sharding: data-parallel over batch across 8 NeuronCores (4 rows per core).

Per core, per batch row (8 column-tiles of 128 tokens; token s = 128j + p):
  1. matvec: fused mult + add-reduce (DVE scalar_tensor_tensor + accum)
     against a broadcast W tile; relu(x+b) on ACT; special ids masked to 0.
  2. per-column duplicate resolution, batched over the whole row as three
     [128, 8, 128] DVE passes using broadcast access patterns:
       eq[p,j,q]  = (id[128j+p] == id[128j+q])
       cnt[p,j]   = sum_q eq * tril[p,q]   -> 0 iff p is its group's first
       gsum[p,j]  = sum_q eq * tw[128j+q]  -> within-column group sum
     Non-carrier tokens (cnt != 0) and zero-sum tokens get their id remapped
     out of bounds so the scatter drops their descriptors: no instruction
     ever contains duplicate offsets (the DMA's within-instruction
     read-modify-write races), and zero-adds cost nothing.
  3. 8 indirect cce-add scatters per row, sequenced by Tile (same-tensor
     WAW), so cross-column duplicates accumulate exactly.
"""

import numpy as np

B, S, H, V = 32, 1024, 1024, 250002
N_CORES = 8
B_LOC = B // N_CORES          # 4 batch rows per core
VPAD = 128 * 1954             # 250112 >= V, divisible by 128
N_STILE = S // 128            # 8 column-tiles per row
BIG = 524288.0                # OOB offset for dropped tokens

ZERO_INIT = True

_compiled = {}


def _build(b_val: float):
    import concourse.bass as bass
    import concourse.tile as tile
    from concourse import bacc, mybir

    f32 = mybir.dt.float32
    i32 = mybir.dt.int32
    Alu = mybir.AluOpType

    nc = bacc.Bacc("TRN2", target_bir_lowering=False, debug=False)

    hs = nc.dram_tensor("hs", [B_LOC, S, H], f32, kind="ExternalInput")
    ids = nc.dram_tensor("ids", [B_LOC, S], i32, kind="ExternalInput")
    w = nc.dram_tensor("w", [1, H], f32, kind="ExternalInput")
    tables = [
        nc.dram_tensor(f"t{r}", [VPAD, 1], f32, kind="ExternalOutput")
        for r in range(B_LOC)
    ]

    with tile.TileContext(nc) as tc:
        with (
            tc.tile_pool(name="const", bufs=1) as const_pool,
            tc.tile_pool(name="h", bufs=6) as h_pool,
            tc.tile_pool(name="big", bufs=2) as big_pool,
            tc.tile_pool(name="sc", bufs=3) as sc_pool,
            tc.tile_pool(name="sm", bufs=2) as sm_pool,
            tc.tile_pool(name="dram", bufs=1, space="DRAM") as dram_pool,
        ):
            # ---- one-time constants ----
            w_bc = const_pool.tile([128, H], f32)
            nc.sync.dma_start(w_bc[:], w[0:1, :].to_broadcast([128, H]))

            ones = const_pool.tile([128, 128], f32)
            nc.vector.memset(ones[:], 1.0)
            tril = const_pool.tile([128, 128], f32)
            # tril[p, q] = 1.0 where p - q > 0 else 0.0
            nc.gpsimd.affine_select(
                out=tril[:],
                in_=ones[:],
                pattern=[[-1, 128]],
                compare_op=Alu.is_gt,
                fill=0.0,
                base=0,
                channel_multiplier=1,
            )

            if ZERO_INIT:
                zt = const_pool.tile([128, VPAD // 128], f32)
                nc.vector.memset(zt[:], 0.0)
                for r in range(B_LOC):
                    dst = tables[r][:].rearrange("(p x) 1 -> p x", p=128)
                    nc.sync.dma_start(dst, zt[:])

            for r in range(B_LOC):
                # ---- matvec: twraw[p, j] = sum_h hs[r, 128j+p, :] * W ----
                twraw = sm_pool.tile([128, N_STILE], f32, tag="twraw")
                for j in range(N_STILE):
                    ht = h_pool.tile([128, H], f32, tag="h")
                    nc.scalar.dma_start(ht[:], hs[r, 128 * j : 128 * (j + 1), :])
                    prod = sc_pool.tile([128, H], f32, tag="prod")
                    nc.vector.scalar_tensor_tensor(
                        out=prod[:],
                        in0=ht[:],
                        scalar=1.0,
                        in1=w_bc[:],
                        op0=Alu.mult,
                        op1=Alu.mult,
                        accum_out=twraw[:, j : j + 1],
                    )

                # ---- relu(x + b); mask special tokens (ids < 4) to 0 ----
                twrelu = sm_pool.tile([128, N_STILE], f32, tag="twrelu")
                nc.scalar.activation(
                    twrelu[:],
                    twraw[:],
                    mybir.ActivationFunctionType.Relu,
                    bias=float(b_val),
                )
                idc_i = sm_pool.tile([128, N_STILE], i32, tag="idc_i")
                nc.sync.dma_start(
                    idc_i[:], ids[r, :].rearrange("(j p) -> p j", p=128)
                )
                idc_f = sm_pool.tile([128, N_STILE], f32, tag="idc_f")
                nc.vector.tensor_copy(idc_f[:], idc_i[:])
                twm = sm_pool.tile([128, N_STILE], f32, tag="twm")
                nc.vector.scalar_tensor_tensor(
                    out=twm[:],
                    in0=idc_f[:],
                    scalar=4.0,
                    in1=twrelu[:],
                    op0=Alu.is_ge,
                    op1=Alu.mult,
                )

                # ---- bounce masked weights to DRAM in s-order; broadcast ----
                scr = dram_pool.tile([S], f32, tag="scr")
                nc.sync.dma_start(scr[:].rearrange("(j p) -> p j", p=128), twm[:])
                twT = big_pool.tile([128, S], f32, tag="twT")
                nc.sync.dma_start(twT[:], scr[:][None, :].to_broadcast([128, S]))
                idT_i = big_pool.tile([128, S], i32, tag="idT_i")
                nc.sync.dma_start(
                    idT_i[:], ids[r, :][None, :].to_broadcast([128, S])
                )
                idT_f = big_pool.tile([128, S], f32, tag="idT_f")
                nc.vector.tensor_copy(idT_f[:], idT_i[:])

                # ---- batched per-column dedup: three [128,8,128] passes ----
                c3 = [128, N_STILE, 128]
                eq8 = big_pool.tile([128, S], f32, tag="eq8")
                eq8_3d = eq8[:].rearrange("p (j q) -> p j q", j=N_STILE)
                nc.vector.tensor_tensor(
                    out=eq8_3d,
                    in0=idc_f[:, :, None].to_broadcast(c3),
                    in1=idT_f[:].rearrange("p (j q) -> p j q", j=N_STILE),
                    op=Alu.is_equal,
                )
                scrt = sc_pool.tile([128, S], f32, tag="scrt")
                scrt_3d = scrt[:].rearrange("p (j q) -> p j q", j=N_STILE)
                cnt8 = sm_pool.tile([128, N_STILE], f32, tag="cnt8")
                nc.vector.tensor_tensor(
                    out=scrt_3d,
                    in0=eq8_3d,
                    in1=tril[:, None, :].to_broadcast(c3),
                    op=Alu.mult,
                )
                nc.vector.reduce_sum(
                    out=cnt8[:], in_=scrt_3d, axis=mybir.AxisListType.X
                )
                scrg = sc_pool.tile([128, S], f32, tag="scrg")
                scrg_3d = scrg[:].rearrange("p (j q) -> p j q", j=N_STILE)
                gsum8 = sm_pool.tile([128, N_STILE], f32, tag="gsum8")
                nc.vector.tensor_tensor(
                    out=scrg_3d,
                    in0=eq8_3d,
                    in1=twT[:].rearrange("p (j q) -> p j q", j=N_STILE),
                    op=Alu.mult,
                )
                nc.vector.reduce_sum(
                    out=gsum8[:], in_=scrg_3d, axis=mybir.AxisListType.X
                )

                # ---- remap non-carriers and zero-sums out of bounds ----
                nb = sm_pool.tile([128, N_STILE], f32, tag="nb")
                nc.vector.tensor_scalar(
                    out=nb[:],
                    in0=cnt8[:],
                    scalar1=0.0,
                    op0=Alu.not_equal,
                    scalar2=BIG,
                    op1=Alu.mult,
                )
                zb = sm_pool.tile([128, N_STILE], f32, tag="zb")
                nc.vector.tensor_scalar(
                    out=zb[:],
                    in0=gsum8[:],
                    scalar1=0.0,
                    op0=Alu.is_equal,
                    scalar2=BIG,
                    op1=Alu.mult,
                )
                idx_f = sm_pool.tile([128, N_STILE], f32, tag="idx_f")
                nc.vector.tensor_tensor(
                    out=idx_f[:], in0=idc_f[:], in1=nb[:], op=Alu.add
                )
                nc.vector.tensor_tensor(
                    out=idx_f[:], in0=idx_f[:], in1=zb[:], op=Alu.add
                )
                idx_i = sm_pool.tile([128, N_STILE], i32, tag="idx_i")
                nc.vector.tensor_copy(idx_i[:], idx_f[:])

                # ---- sequenced cce-add scatters (<=128 offsets each) ----
                for j in range(N_STILE):
                    nc.gpsimd.indirect_dma_start(
                        out=tables[r][:],
                        out_offset=bass.IndirectOffsetOnAxis(
                            ap=idx_i[:, j : j + 1], axis=0
                        ),
                        in_=gsum8[:, j : j + 1],
                        in_offset=None,
                        compute_op=Alu.add,
                        bounds_check=V - 1,
                        oob_is_err=False,
                    )

    nc.compile()
    return nc


def _get_nc(b_val: float):
    key = float(b_val)
    if key not in _compiled:
        _compiled[key] = _build(key)
    return _compiled[key]


def kernel(hidden_state, input_ids, W_sparse, b_sparse):
    from concourse.bass_utils import run_bass_kernel_spmd

    hidden_state = np.ascontiguousarray(np.asarray(hidden_state, dtype=np.float32))
    input_ids = np.ascontiguousarray(np.asarray(input_ids, dtype=np.int32))
    W_sparse = np.ascontiguousarray(np.asarray(W_sparse, dtype=np.float32))
    b_val = float(np.asarray(b_sparse).reshape(-1)[0])

    nc = _get_nc(b_val)

    in_maps = []
    for c in range(N_CORES):
        sl = slice(c * B_LOC, (c + 1) * B_LOC)
        in_maps.append(
            {"hs": hidden_state[sl], "ids": input_ids[sl], "w": W_sparse}
        )

    res = run_bass_kernel_spmd(nc, in_maps, list(range(N_CORES)))

    out = np.empty((B, V), dtype=np.float32)
    for c in range(N_CORES):
        for r in range(B_LOC):
            out[c * B_LOC + r] = res.results[c][f"t{r}"][:V, 0]
    return out


# revision 13
# speedup vs baseline: 1.2823x; 1.2823x over previous
"""Trainium2 Bass kernel for BGEM3 sparse-embedding head (segment_reduce).

Computes, for inputs hidden_state [B,S,H], input_ids [B,S], W_sparse [1,H],
b_sparse [1]:
    tw = relu(hidden_state @ W_sparse[0] + b_sparse[0])          # [B,S]
    out = zeros([B,V]); out.at[b, ids].max(tw)  (jax scatter-max, which on
    this stack sums duplicate indices); out[:, 0:4] = 0
Sharding: data-parallel over batch across 8 NeuronCores (4 rows per core).

Per core, per batch row (8 column-tiles of 128 tokens; token s = 128j + p):
  1. matvec: fused mult + add-reduce (DVE scalar_tensor_tensor + accum)
     against a broadcast W tile; relu(x+b) on ACT; special ids masked to 0.
  2. per-column duplicate resolution:
       eq[p,j,q] = (id[128j+p] == id[128j+q])   one [128,8,128] DVE pass
       cnt[p,j]  = sum_q eq * tril[p,q]         mult + reduce (DVE)
       gsum[:,j] = eq_j @ twm[:,j]              one PE matmul per column
     Tokens that are not their column's first occurrence (cnt != 0) or have
     zero sum get ids remapped out of bounds -> the scatter drops their
     descriptors, so no instruction carries duplicate offsets (the DMA's
     within-instruction read-modify-write races).
  3. 8 indirect cce-add scatters per row (one offset per partition is a
     hardware limit), sequenced by Tile via same-table WAW, so cross-column
     duplicates accumulate exactly.
"""

import numpy as np

B, S, H, V = 32, 1024, 1024, 250002
N_CORES = 8
B_LOC = B // N_CORES          # 4 batch rows per core
VPAD = 128 * 1954             # 250112 >= V, divisible by 128
N_STILE = S // 128            # 8 column-tiles per row
BIG = 524288.0                # OOB offset for dropped tokens

ZERO_INIT = True

_compiled = {}


def _build(b_val: float):
    import concourse.bass as bass
    import concourse.tile as tile
    from concourse import bacc, mybir

    f32 = mybir.dt.float32
    i32 = mybir.dt.int32
    Alu = mybir.AluOpType

    nc = bacc.Bacc("TRN2", target_bir_lowering=False, debug=False)

    hs = nc.dram_tensor("hs", [B_LOC, S, H], f32, kind="ExternalInput")
    ids = nc.dram_tensor("ids", [B_LOC, S], i32, kind="ExternalInput")
    w = nc.dram_tensor("w", [1, H], f32, kind="ExternalInput")
    tables = [
        nc.dram_tensor(f"t{r}", [VPAD, 1], f32, kind="ExternalOutput")
        for r in range(B_LOC)
    ]

    with tile.TileContext(nc) as tc:
        with (
            tc.tile_pool(name="const", bufs=1) as const_pool,
            tc.tile_pool(name="h", bufs=6) as h_pool,
            tc.tile_pool(name="eq", bufs=2) as eq_pool,
            tc.tile_pool(name="bc", bufs=2) as bc_pool,
            tc.tile_pool(name="sc", bufs=2) as sc_pool,
            tc.tile_pool(name="sm", bufs=2) as sm_pool,
            tc.tile_pool(name="ps", bufs=2, space="PSUM") as ps_pool,
        ):
            # ---- one-time constants ----
            w_bc = const_pool.tile([128, H], f32)
            nc.sync.dma_start(w_bc[:], w[0:1, :].to_broadcast([128, H]))

            ones = const_pool.tile([128, 128], f32)
            nc.vector.memset(ones[:], 1.0)
            # tril[p, q] = 1.0 where q < p
            tril = const_pool.tile([128, 128], f32)
            nc.gpsimd.affine_select(
                out=tril[:],
                in_=ones[:],
                pattern=[[-1, 128]],
                compare_op=Alu.is_gt,
                fill=0.0,
                base=0,
                channel_multiplier=1,
            )

            if ZERO_INIT:
                zt = const_pool.tile([128, VPAD // 128], f32)
                nc.vector.memset(zt[:], 0.0)
                for r in range(B_LOC):
                    dst = tables[r][:].rearrange("(p x) 1 -> p x", p=128)
                    nc.sync.dma_start(dst, zt[:])

            # ---- id loads for all rows up front (small DMAs) ----
            idc_i, idc_f, idT_i = {}, {}, {}
            for r in range(B_LOC):
                idc_i[r] = sm_pool.tile(
                    [128, N_STILE], i32, tag=f"idc_i{r}", name=f"idc_i{r}"
                )
                nc.sync.dma_start(
                    idc_i[r][:], ids[r, :].rearrange("(j p) -> p j", p=128)
                )
                idT_i[r] = bc_pool.tile(
                    [128, S], i32, tag=f"idT_i{r}", name=f"idT_i{r}"
                )
                nc.sync.dma_start(
                    idT_i[r][:], ids[r, :][None, :].to_broadcast([128, S])
                )

            c3 = [128, N_STILE, 128]
            for r in range(B_LOC):
                # ---- matvec ----
                twraw = sm_pool.tile([128, N_STILE], f32, tag="twraw")
                for j in range(N_STILE):
                    ht = h_pool.tile([128, H], f32, tag="h")
                    nc.scalar.dma_start(ht[:], hs[r, 128 * j : 128 * (j + 1), :])
                    prod = sc_pool.tile([128, H], f32, tag="prod")
                    nc.vector.scalar_tensor_tensor(
                        out=prod[:],
                        in0=ht[:],
                        scalar=1.0,
                        in1=w_bc[:],
                        op0=Alu.mult,
                        op1=Alu.mult,
                        accum_out=twraw[:, j : j + 1],
                    )

                # ---- relu(x + b), special-token mask ----
                idc_fr = sm_pool.tile([128, N_STILE], f32, tag="idc_f")
                nc.vector.tensor_copy(idc_fr[:], idc_i[r][:])
                twrelu = sm_pool.tile([128, N_STILE], f32, tag="twrelu")
                nc.scalar.activation(
                    twrelu[:],
                    twraw[:],
                    mybir.ActivationFunctionType.Relu,
                    bias=float(b_val),
                )
                twm = sm_pool.tile([128, N_STILE], f32, tag="twm")
                nc.vector.scalar_tensor_tensor(
                    out=twm[:],
                    in0=idc_fr[:],
                    scalar=4.0,
                    in1=twrelu[:],
                    op0=Alu.is_ge,
                    op1=Alu.mult,
                )

                # ---- per-column eq matrix + prior-duplicate count ----
                idT_f = bc_pool.tile([128, S], f32, tag="idT_f")
                nc.vector.tensor_copy(idT_f[:], idT_i[r][:])
                eq8 = eq_pool.tile([128, S], f32, tag="eq8")
                eq_3d = eq8[:].rearrange("p (j q) -> p j q", j=N_STILE)
                nc.vector.tensor_tensor(
                    out=eq_3d,
                    in0=idc_fr[:, :, None].to_broadcast(c3),
                    in1=idT_f[:].rearrange("p (j q) -> p j q", j=N_STILE),
                    op=Alu.is_equal,
                )
                scrt = sc_pool.tile([128, S], f32, tag="scrt")
                scrt_3d = scrt[:].rearrange("p (j q) -> p j q", j=N_STILE)
                nc.vector.tensor_tensor(
                    out=scrt_3d,
                    in0=eq_3d,
                    in1=tril[:, None, :].to_broadcast(c3),
                    op=Alu.mult,
                )
                cnt8 = sm_pool.tile([128, N_STILE], f32, tag="cnt8")
                nc.vector.reduce_sum(
                    out=cnt8[:], in_=scrt_3d, axis=mybir.AxisListType.X
                )

                # ---- gsum[:, j] = eq_j @ twm[:, j] on PE ----
                gsum_ps = ps_pool.tile([128, N_STILE], f32, tag="gsum")
                for j in range(N_STILE):
                    nc.tensor.matmul(
                        out=gsum_ps[:, j : j + 1],
                        lhsT=eq8[:, 128 * j : 128 * (j + 1)],
                        rhs=twm[:, j : j + 1],
                        start=True,
                        stop=True,
                    )
                gsum = sm_pool.tile([128, N_STILE], f32, tag="gsumsb")
                nc.vector.tensor_copy(gsum[:], gsum_ps[:])

                # ---- remap dropped tokens out of bounds ----
                nb = sm_pool.tile([128, N_STILE], f32, tag="nb")
                nc.vector.tensor_scalar(
                    out=nb[:],
                    in0=cnt8[:],
                    scalar1=0.0,
                    op0=Alu.not_equal,
                    scalar2=BIG,
                    op1=Alu.mult,
                )
                zb = sm_pool.tile([128, N_STILE], f32, tag="zb")
                nc.vector.tensor_scalar(
                    out=zb[:],
                    in0=gsum[:],
                    scalar1=0.0,
                    op0=Alu.is_equal,
                    scalar2=BIG,
                    op1=Alu.mult,
                )
                idx_f = sm_pool.tile([128, N_STILE], f32, tag="idx_f")
                nc.vector.tensor_tensor(
                    out=idx_f[:], in0=idc_fr[:], in1=nb[:], op=Alu.add
                )
                nc.vector.tensor_tensor(
                    out=idx_f[:], in0=idx_f[:], in1=zb[:], op=Alu.add
                )
                idx_i = sm_pool.tile([128, N_STILE], i32, tag="idx_i")
                nc.vector.tensor_copy(idx_i[:], idx_f[:])

                # ---- sequenced cce-add scatters ----
                for j in range(N_STILE):
                    nc.gpsimd.indirect_dma_start(
                        out=tables[r][:],
                        out_offset=bass.IndirectOffsetOnAxis(
                            ap=idx_i[:, j : j + 1], axis=0
                        ),
                        in_=gsum[:, j : j + 1],
                        in_offset=None,
                        compute_op=Alu.add,
                        bounds_check=V - 1,
                        oob_is_err=False,
                    )

    nc.compile()
    return nc


def _get_nc(b_val: float):
    key = float(b_val)
    if key not in _compiled:
        _compiled[key] = _build(key)
    return _compiled[key]


def kernel(hidden_state, input_ids, W_sparse, b_sparse):
    from concourse.bass_utils import run_bass_kernel_spmd

    hidden_state = np.ascontiguousarray(np.asarray(hidden_state, dtype=np.float32))
    input_ids = np.ascontiguousarray(np.asarray(input_ids, dtype=np.int32))
    W_sparse = np.ascontiguousarray(np.asarray(W_sparse, dtype=np.float32))
    b_val = float(np.asarray(b_sparse).reshape(-1)[0])

    nc = _get_nc(b_val)

    in_maps = []
    for c in range(N_CORES):
        sl = slice(c * B_LOC, (c + 1) * B_LOC)
        in_maps.append(
            {"hs": hidden_state[sl], "ids": input_ids[sl], "w": W_sparse}
        )

    res = run_bass_kernel_spmd(nc, in_maps, list(range(N_CORES)))

    out = np.empty((B, V), dtype=np.float32)
    for c in range(N_CORES):
        for r in range(B_LOC):
            out[c * B_LOC + r] = res.results[c][f"t{r}"][:V, 0]
    return out


# revision 14
# speedup vs baseline: 1.3070x; 1.0192x over previous
"""Trainium2 Bass kernel for BGEM3 sparse-embedding head (segment_reduce).

Computes, for inputs hidden_state [B,S,H], input_ids [B,S], W_sparse [1,H],
b_sparse [1]:
    tw = relu(hidden_state @ W_sparse[0] + b_sparse[0])          # [B,S]
    out = zeros([B,V]); out.at[b, ids].max(tw)  (jax scatter-max, which on
    this stack sums duplicate indices); out[:, 0:4] = 0
Sharding: data-parallel over batch across 8 NeuronCores (4 rows per core).

Per core, per batch row (8 column-tiles of 128 tokens; token s = 128j + p):
  1. matvec: fused mult + add-reduce (DVE scalar_tensor_tensor + accum)
     against a broadcast W tile; relu(x+b) on ACT; special ids masked to 0.
  2. per-column duplicate resolution:
       eq[p,j,q] = (id[128j+p] == id[128j+q])   one [128,8,128] DVE pass
       cnt[p,j]  = sum_q eq * tril[p,q]         mult + reduce (DVE)
       gsum[:,j] = eq_j @ twm[:,j]              one PE matmul per column
     Tokens that are not their column's first occurrence (cnt != 0) or have
     zero sum get ids remapped out of bounds -> the scatter drops their
     descriptors, so no instruction carries duplicate offsets (the DMA's
     within-instruction read-modify-write races).
  3. 8 indirect cce-add scatters per row (one offset per partition is a
     hardware limit), sequenced by Tile via same-table WAW, so cross-column
     duplicates accumulate exactly.
"""

import numpy as np

B, S, H, V = 32, 1024, 1024, 250002
N_CORES = 8
B_LOC = B // N_CORES          # 4 batch rows per core
VPAD = 128 * 1954             # 250112 >= V, divisible by 128
N_STILE = S // 128            # 8 column-tiles per row
BIG = 524288.0                # OOB offset for dropped tokens

ZERO_INIT = False

_compiled = {}


def _build(b_val: float):
    import concourse.bass as bass
    import concourse.tile as tile
    from concourse import bacc, mybir

    f32 = mybir.dt.float32
    i32 = mybir.dt.int32
    Alu = mybir.AluOpType

    nc = bacc.Bacc("TRN2", target_bir_lowering=False, debug=False)

    hs = nc.dram_tensor("hs", [B_LOC, S, H], f32, kind="ExternalInput")
    ids = nc.dram_tensor("ids", [B_LOC, S], i32, kind="ExternalInput")
    w = nc.dram_tensor("w", [1, H], f32, kind="ExternalInput")
    tables = [
        nc.dram_tensor(f"t{r}", [VPAD, 1], f32, kind="ExternalOutput")
        for r in range(B_LOC)
    ]

    with tile.TileContext(nc) as tc:
        with (
            tc.tile_pool(name="const", bufs=1) as const_pool,
            tc.tile_pool(name="h", bufs=6) as h_pool,
            tc.tile_pool(name="eq", bufs=2) as eq_pool,
            tc.tile_pool(name="bc", bufs=2) as bc_pool,
            tc.tile_pool(name="sc", bufs=2) as sc_pool,
            tc.tile_pool(name="sm", bufs=2) as sm_pool,
            tc.tile_pool(name="ps", bufs=2, space="PSUM") as ps_pool,
        ):
            # ---- one-time constants ----
            w_bc = const_pool.tile([128, H], f32)
            nc.sync.dma_start(w_bc[:], w[0:1, :].to_broadcast([128, H]))

            ones = const_pool.tile([128, 128], f32)
            nc.vector.memset(ones[:], 1.0)
            # tril[p, q] = 1.0 where q < p
            tril = const_pool.tile([128, 128], f32)
            nc.gpsimd.affine_select(
                out=tril[:],
                in_=ones[:],
                pattern=[[-1, 128]],
                compare_op=Alu.is_gt,
                fill=0.0,
                base=0,
                channel_multiplier=1,
            )

            if ZERO_INIT:
                zt = const_pool.tile([128, VPAD // 128], f32)
                nc.vector.memset(zt[:], 0.0)
                for r in range(B_LOC):
                    dst = tables[r][:].rearrange("(p x) 1 -> p x", p=128)
                    nc.sync.dma_start(dst, zt[:])

            # ---- id loads for all rows up front (small DMAs) ----
            idc_i, idc_f, idT_i = {}, {}, {}
            for r in range(B_LOC):
                idc_i[r] = sm_pool.tile(
                    [128, N_STILE], i32, tag=f"idc_i{r}", name=f"idc_i{r}"
                )
                nc.sync.dma_start(
                    idc_i[r][:], ids[r, :].rearrange("(j p) -> p j", p=128)
                )
                idT_i[r] = bc_pool.tile(
                    [128, S], i32, tag=f"idT_i{r}", name=f"idT_i{r}"
                )
                nc.sync.dma_start(
                    idT_i[r][:], ids[r, :][None, :].to_broadcast([128, S])
                )

            gsum_t, idx_t = {}, {}

            def emit_scatters(pairs):
                for r_, j_ in pairs:
                    nc.gpsimd.indirect_dma_start(
                        out=tables[r_][:],
                        out_offset=bass.IndirectOffsetOnAxis(
                            ap=idx_t[r_][:, j_ : j_ + 1], axis=0
                        ),
                        in_=gsum_t[r_][:, j_ : j_ + 1],
                        in_offset=None,
                        compute_op=Alu.add,
                        bounds_check=V - 1,
                        oob_is_err=False,
                    )

            c3 = [128, N_STILE, 128]
            for r in range(B_LOC):
                # ---- matvec ----
                twraw = sm_pool.tile([128, N_STILE], f32, tag="twraw")
                for j in range(N_STILE):
                    ht = h_pool.tile([128, H], f32, tag="h")
                    nc.scalar.dma_start(ht[:], hs[r, 128 * j : 128 * (j + 1), :])
                    prod = sc_pool.tile([128, H], f32, tag="prod")
                    nc.vector.scalar_tensor_tensor(
                        out=prod[:],
                        in0=ht[:],
                        scalar=1.0,
                        in1=w_bc[:],
                        op0=Alu.mult,
                        op1=Alu.mult,
                        accum_out=twraw[:, j : j + 1],
                    )

                # ---- relu(x + b), special-token mask ----
                idc_fr = sm_pool.tile([128, N_STILE], f32, tag="idc_f")
                nc.vector.tensor_copy(idc_fr[:], idc_i[r][:])
                twrelu = sm_pool.tile([128, N_STILE], f32, tag="twrelu")
                nc.scalar.activation(
                    twrelu[:],
                    twraw[:],
                    mybir.ActivationFunctionType.Relu,
                    bias=float(b_val),
                )
                twm = sm_pool.tile([128, N_STILE], f32, tag="twm")
                nc.vector.scalar_tensor_tensor(
                    out=twm[:],
                    in0=idc_fr[:],
                    scalar=4.0,
                    in1=twrelu[:],
                    op0=Alu.is_ge,
                    op1=Alu.mult,
                )

                # ---- per-column eq matrix + prior-duplicate count ----
                idT_f = bc_pool.tile([128, S], f32, tag="idT_f")
                nc.vector.tensor_copy(idT_f[:], idT_i[r][:])
                eq8 = eq_pool.tile([128, S], f32, tag="eq8")
                eq_3d = eq8[:].rearrange("p (j q) -> p j q", j=N_STILE)
                nc.vector.tensor_tensor(
                    out=eq_3d,
                    in0=idc_fr[:, :, None].to_broadcast(c3),
                    in1=idT_f[:].rearrange("p (j q) -> p j q", j=N_STILE),
                    op=Alu.is_equal,
                )
                scrt = sc_pool.tile([128, S], f32, tag="scrt")
                scrt_3d = scrt[:].rearrange("p (j q) -> p j q", j=N_STILE)
                nc.vector.tensor_tensor(
                    out=scrt_3d,
                    in0=eq_3d,
                    in1=tril[:, None, :].to_broadcast(c3),
                    op=Alu.mult,
                )
                cnt8 = sm_pool.tile([128, N_STILE], f32, tag="cnt8")
                nc.vector.reduce_sum(
                    out=cnt8[:], in_=scrt_3d, axis=mybir.AxisListType.X
                )

                # ---- gsum[:, j] = eq_j @ twm[:, j] on PE ----
                gsum_ps = ps_pool.tile([128, N_STILE], f32, tag="gsum")
                for j in range(N_STILE):
                    nc.tensor.matmul(
                        out=gsum_ps[:, j : j + 1],
                        lhsT=eq8[:, 128 * j : 128 * (j + 1)],
                        rhs=twm[:, j : j + 1],
                        start=True,
                        stop=True,
                    )
                gsum = sm_pool.tile(
                    [128, N_STILE], f32, tag=f"gsumsb{r}", name=f"gsumsb{r}"
                )
                nc.vector.tensor_copy(gsum[:], gsum_ps[:])
                gsum_t[r] = gsum

                # ---- remap dropped tokens out of bounds ----
                nb = sm_pool.tile([128, N_STILE], f32, tag="nb")
                nc.vector.tensor_scalar(
                    out=nb[:],
                    in0=cnt8[:],
                    scalar1=0.0,
                    op0=Alu.not_equal,
                    scalar2=BIG,
                    op1=Alu.mult,
                )
                zb = sm_pool.tile([128, N_STILE], f32, tag="zb")
                nc.vector.tensor_scalar(
                    out=zb[:],
                    in0=gsum[:],
                    scalar1=0.0,
                    op0=Alu.is_equal,
                    scalar2=BIG,
                    op1=Alu.mult,
                )
                idx_f = sm_pool.tile([128, N_STILE], f32, tag="idx_f")
                nc.vector.tensor_tensor(
                    out=idx_f[:], in0=idc_fr[:], in1=nb[:], op=Alu.add
                )
                nc.vector.tensor_tensor(
                    out=idx_f[:], in0=idx_f[:], in1=zb[:], op=Alu.add
                )
                idx_i = sm_pool.tile(
                    [128, N_STILE], i32, tag=f"idx_i{r}", name=f"idx_i{r}"
                )
                nc.vector.tensor_copy(idx_i[:], idx_f[:])
                idx_t[r] = idx_i

                if r == 0:
                    # row 0's chain starts as soon as its data is ready; its
                    # completion waits overlap rows 1-3's matvec.
                    emit_scatters([(0, j) for j in range(N_STILE)])

            # Rows 1-3 interleaved column-major: each row's next scatter is
            # spaced 3 instructions apart, hiding the same-table completion
            # wait behind the other rows' descriptor generation.
            emit_scatters(
                [(r, j) for j in range(N_STILE) for r in range(1, B_LOC)]
            )

    nc.compile()
    return nc


def _get_nc(b_val: float):
    key = float(b_val)
    if key not in _compiled:
        _compiled[key] = _build(key)
    return _compiled[key]


def kernel(hidden_state, input_ids, W_sparse, b_sparse):
    from concourse.bass_utils import run_bass_kernel_spmd

    hidden_state = np.ascontiguousarray(np.asarray(hidden_state, dtype=np.float32))
    input_ids = np.ascontiguousarray(np.asarray(input_ids, dtype=np.int32))
    W_sparse = np.ascontiguousarray(np.asarray(W_sparse, dtype=np.float32))
    b_val = float(np.asarray(b_sparse).reshape(-1)[0])

    nc = _get_nc(b_val)

    in_maps = []
    for c in range(N_CORES):
        sl = slice(c * B_LOC, (c + 1) * B_LOC)
        in_maps.append(
            {"hs": hidden_state[sl], "ids": input_ids[sl], "w": W_sparse}
        )

    res = run_bass_kernel_spmd(nc, in_maps, list(range(N_CORES)))

    out = np.empty((B, V), dtype=np.float32)
    for c in range(N_CORES):
        for r in range(B_LOC):
            out[c * B_LOC + r] = res.results[c][f"t{r}"][:V, 0]
    return out


# revision 18
# speedup vs baseline: 1.8309x; 1.4008x over previous
"""Trainium2 Bass kernel for BGEM3 sparse-embedding head (segment_reduce).

Computes, for inputs hidden_state [B,S,H], input_ids [B,S], W_sparse [1,H],
b_sparse [1]:
    tw = relu(hidden_state @ W_sparse[0] + b_sparse[0])          # [B,S]
    out = zeros([B,V]); out.at[b, ids].max(tw)  (jax scatter-max, which on
    this stack sums duplicate indices); out[:, 0:4] = 0
Sharding: data-parallel over batch across 8 NeuronCores (4 rows per core).

Per core, per batch row (8 column-tiles of 128 tokens; token s = 128j + p):
  1. matvec: fused mult + add-reduce (DVE scalar_tensor_tensor + accum)
     against a W tile broadcast via PE (ones outer product); relu(x+b) on
     ACT; special ids (< 4) masked to 0.
  2. eq[p,j,q] = (id[128j+p] == id[128j+q]): ids transposed per column on PE
     (broadcast-transpose into PSUM), one batched [128,8,128] DVE pass.
     gsum[:, j] = eq_j @ twm[:, j] (one PE matmul per column): every token
     gets its within-column duplicate-group sum.
  3. Column j of row r scatters (plain writes, 128 offsets = one per
     partition, a hardware limit) into its OWN table section at offset
     j*VPAD. Within a column, duplicate offsets all carry the identical
     group sum, so collisions are benign; across columns the sections are
     disjoint, so no ordering constraints exist anywhere -> the 32 scatters
     stream back-to-back on GpSimd. The host sums the 8 sections per row
     while unsharding (the standard gather for a sum-sharded output).
Output tables rely on the runtime's zero-donated output buffers
(run_bass_via_pjrt donates np.zeros as the ExternalOutput backing).
"""

import numpy as np

B, S, H, V = 32, 1024, 1024, 250002
N_CORES = 8
B_LOC = B // N_CORES          # 4 batch rows per core
VPAD = 128 * 1954             # 250112 >= V, divisible by 128
N_STILE = S // 128            # 8 column-tiles per row

_compiled = {}


def _build(b_val: float):
    import concourse.bass as bass
    import concourse.tile as tile
    from concourse import bacc, mybir
    from concourse.masks import make_identity

    f32 = mybir.dt.float32
    i32 = mybir.dt.int32
    Alu = mybir.AluOpType

    nc = bacc.Bacc("TRN2", target_bir_lowering=False, debug=False)

    hs = nc.dram_tensor("hs", [B_LOC, S, H], f32, kind="ExternalInput")
    ids = nc.dram_tensor("ids", [B_LOC, S], i32, kind="ExternalInput")
    w = nc.dram_tensor("w", [1, H], f32, kind="ExternalInput")
    tables = [
        nc.dram_tensor(f"t{r}", [N_STILE * VPAD, 1], f32, kind="ExternalOutput")
        for r in range(B_LOC)
    ]

    with tile.TileContext(nc) as tc:
        with (
            tc.tile_pool(name="const", bufs=1) as const_pool,
            tc.tile_pool(name="h", bufs=6) as h_pool,
            tc.tile_pool(name="eq", bufs=2) as eq_pool,
            tc.tile_pool(name="sc", bufs=2) as sc_pool,
            tc.tile_pool(name="sm", bufs=2) as sm_pool,
            tc.tile_pool(name="ps", bufs=2, space="PSUM") as ps_pool,
        ):
            # ---- W broadcast to 128 partitions via PE outer product ----
            ones1 = const_pool.tile([1, 128], f32)
            nc.vector.memset(ones1[:], 1.0)
            w_row = const_pool.tile([1, H], f32)
            nc.sync.dma_start(w_row[:], w[0:1, :])
            wb_ps = ps_pool.tile([128, H], f32, tag="wb_ps", bufs=1)
            for half in range(2):
                sl = slice(512 * half, 512 * (half + 1))
                nc.tensor.matmul(
                    out=wb_ps[:, sl],
                    lhsT=ones1[:],
                    rhs=w_row[:, sl],
                    start=True,
                    stop=True,
                )
            w_bc = const_pool.tile([128, H], f32)
            nc.scalar.copy(w_bc[:], wb_ps[:])

            ident = const_pool.tile([128, 128], f32)
            make_identity(nc, ident[:])

            # ---- all rows' ids in one tile: (p, r, j) = ids[r, 128j+p] ----
            idc_all = sm_pool.tile([128, B_LOC, N_STILE], i32, bufs=1)
            nc.sync.dma_start(
                idc_all[:], ids[:, :].rearrange("r (j p) -> p r j", p=128)
            )
            idc_f_all = sm_pool.tile([128, B_LOC, N_STILE], f32, bufs=1)
            nc.scalar.copy(idc_f_all[:], idc_all[:])

            c3 = [128, N_STILE, 128]
            for r in range(B_LOC):
                # ---- matvec ----
                twraw = sm_pool.tile([128, N_STILE], f32, tag="twraw")
                for j in range(N_STILE):
                    ht = h_pool.tile([128, H], f32, tag="h")
                    nc.sync.dma_start(ht[:], hs[r, 128 * j : 128 * (j + 1), :])
                    prod = sc_pool.tile([128, H], f32, tag="prod")
                    nc.vector.scalar_tensor_tensor(
                        out=prod[:],
                        in0=ht[:],
                        scalar=1.0,
                        in1=w_bc[:],
                        op0=Alu.mult,
                        op1=Alu.mult,
                        accum_out=twraw[:, j : j + 1],
                    )

                # ---- per-column transposed ids on PE ----
                idc_fr = idc_f_all[:, r]
                idT_ps = ps_pool.tile([128, S], f32, tag="idT_ps", bufs=1)
                for j in range(N_STILE):
                    nc.tensor.transpose(
                        out=idT_ps[:, 128 * j : 128 * (j + 1)],
                        in_=idc_fr[:, j : j + 1].to_broadcast([128, 128]),
                        identity=ident[:],
                    )

                # ---- relu(x + b), special-token mask ----
                twrelu = sm_pool.tile([128, N_STILE], f32, tag="twrelu")
                nc.scalar.activation(
                    twrelu[:],
                    twraw[:],
                    mybir.ActivationFunctionType.Relu,
                    bias=float(b_val),
                )
                twm = sm_pool.tile([128, N_STILE], f32, tag="twm")
                nc.vector.scalar_tensor_tensor(
                    out=twm[:],
                    in0=idc_fr[:, :],
                    scalar=4.0,
                    in1=twrelu[:],
                    op0=Alu.is_ge,
                    op1=Alu.mult,
                )

                # ---- eq + per-column duplicate-group sums ----
                eq8 = eq_pool.tile([128, S], f32, tag="eq8")
                nc.vector.tensor_tensor(
                    out=eq8[:].rearrange("p (j q) -> p j q", j=N_STILE),
                    in0=idc_fr[:, :, None].to_broadcast(c3),
                    in1=idT_ps[:].rearrange("p (j q) -> p j q", j=N_STILE),
                    op=Alu.is_equal,
                )
                gsum_ps = ps_pool.tile([128, N_STILE], f32, tag="gsum")
                for j in range(N_STILE):
                    nc.tensor.matmul(
                        out=gsum_ps[:, j : j + 1],
                        lhsT=eq8[:, 128 * j : 128 * (j + 1)],
                        rhs=twm[:, j : j + 1],
                        start=True,
                        stop=True,
                    )
                gsum = sm_pool.tile([128, N_STILE], f32, tag="gsumsb")
                nc.scalar.copy(gsum[:], gsum_ps[:])

                # ---- independent bypass scatters, one table section each ----
                for j in range(N_STILE):
                    nc.gpsimd.indirect_dma_start(
                        out=tables[r][:],
                        out_offset=bass.IndirectOffsetOnAxis(
                            ap=idc_all[:, r, j : j + 1], axis=0
                        ),
                        in_=gsum[:, j : j + 1],
                        in_offset=None,
                        element_offset=j * VPAD,
                    )

    nc.compile()
    return nc


def _get_nc(b_val: float):
    key = float(b_val)
    if key not in _compiled:
        _compiled[key] = _build(key)
    return _compiled[key]


def kernel(hidden_state, input_ids, W_sparse, b_sparse):
    from concourse.bass_utils import run_bass_kernel_spmd

    hidden_state = np.ascontiguousarray(np.asarray(hidden_state, dtype=np.float32))
    input_ids = np.ascontiguousarray(np.asarray(input_ids, dtype=np.int32))
    W_sparse = np.ascontiguousarray(np.asarray(W_sparse, dtype=np.float32))
    b_val = float(np.asarray(b_sparse).reshape(-1)[0])

    nc = _get_nc(b_val)

    in_maps = []
    for c in range(N_CORES):
        sl = slice(c * B_LOC, (c + 1) * B_LOC)
        in_maps.append(
            {"hs": hidden_state[sl], "ids": input_ids[sl], "w": W_sparse}
        )

    res = run_bass_kernel_spmd(nc, in_maps, list(range(N_CORES)))

    out = np.empty((B, V), dtype=np.float32)
    for c in range(N_CORES):
        for r in range(B_LOC):
            t = res.results[c][f"t{r}"][:, 0].reshape(N_STILE, VPAD)
            out[c * B_LOC + r] = t[:, :V].sum(axis=0, dtype=np.float32)
    return out


# revision 19
# speedup vs baseline: 2.2100x; 1.2071x over previous
"""Trainium2 Bass kernel for BGEM3 sparse-embedding head (segment_reduce).

Computes, for inputs hidden_state [B,S,H], input_ids [B,S], W_sparse [1,H],
b_sparse [1]:
    tw = relu(hidden_state @ W_sparse[0] + b_sparse[0])          # [B,S]
    out = zeros([B,V]); out.at[b, ids].max(tw)  (jax scatter-max, which on
    this stack sums duplicate indices); out[:, 0:4] = 0
Sharding: data-parallel over batch across 8 NeuronCores (4 rows per core).

Per core, per batch row (8 column-tiles of 128 tokens; token s = 128j + p):
  1. matvec: fused mult + add-reduce (DVE scalar_tensor_tensor + accum)
     against a W tile broadcast via PE (ones outer product); relu(x+b) on
     ACT; special ids (< 4) masked to 0.
  2. eq[p,j,q] = (id[128j+p] == id[128j+q]): ids transposed per column on PE
     (broadcast-transpose into PSUM), one batched [128,8,128] DVE pass.
     gsum[:, j] = eq_j @ twm[:, j] (one PE matmul per column): every token
     gets its within-column duplicate-group sum.
  3. Column j of row r scatters (plain writes, 128 offsets = one per
     partition, a hardware limit) into its OWN table section at offset
     j*VPAD. Within a column, duplicate offsets all carry the identical
     group sum, so collisions are benign; across columns the sections are
     disjoint, so no ordering constraints exist anywhere -> the 32 scatters
     stream back-to-back on GpSimd. The host sums the 8 sections per row
     while unsharding (the standard gather for a sum-sharded output).
Output tables rely on the runtime's zero-donated output buffers
(run_bass_via_pjrt donates np.zeros as the ExternalOutput backing).
"""

import numpy as np

B, S, H, V = 32, 1024, 1024, 250002
N_CORES = 8
B_LOC = B // N_CORES          # 4 batch rows per core
VPAD = 128 * 1954             # 250112 >= V, divisible by 128
N_STILE = S // 128            # 8 column-tiles per row

_compiled = {}


def _build(b_val: float):
    import concourse.bass as bass
    import concourse.tile as tile
    from concourse import bacc, mybir
    from concourse.masks import make_identity

    f32 = mybir.dt.float32
    i32 = mybir.dt.int32
    Alu = mybir.AluOpType

    nc = bacc.Bacc("TRN2", target_bir_lowering=False, debug=False)

    hs = nc.dram_tensor("hs", [B_LOC, S, H], f32, kind="ExternalInput")
    ids = nc.dram_tensor("ids", [B_LOC, S], i32, kind="ExternalInput")
    w = nc.dram_tensor("w", [1, H], f32, kind="ExternalInput")
    tables = [
        [
            nc.dram_tensor(f"t{r}_{j}", [VPAD, 1], f32, kind="ExternalOutput")
            for j in range(N_STILE)
        ]
        for r in range(B_LOC)
    ]

    with tile.TileContext(nc) as tc:
        with (
            tc.tile_pool(name="const", bufs=1) as const_pool,
            tc.tile_pool(name="h", bufs=6) as h_pool,
            tc.tile_pool(name="eq", bufs=2) as eq_pool,
            tc.tile_pool(name="sc", bufs=2) as sc_pool,
            tc.tile_pool(name="sm", bufs=2) as sm_pool,
            tc.tile_pool(name="ps", bufs=2, space="PSUM") as ps_pool,
        ):
            # ---- W broadcast to 128 partitions via PE outer product ----
            ones1 = const_pool.tile([1, 128], f32)
            nc.vector.memset(ones1[:], 1.0)
            w_row = const_pool.tile([1, H], f32)
            nc.sync.dma_start(w_row[:], w[0:1, :])
            wb_ps = ps_pool.tile([128, H], f32, tag="wb_ps", bufs=1)
            for half in range(2):
                sl = slice(512 * half, 512 * (half + 1))
                nc.tensor.matmul(
                    out=wb_ps[:, sl],
                    lhsT=ones1[:],
                    rhs=w_row[:, sl],
                    start=True,
                    stop=True,
                )
            w_bc = const_pool.tile([128, H], f32)
            nc.scalar.copy(w_bc[:], wb_ps[:])

            ident = const_pool.tile([128, 128], f32)
            make_identity(nc, ident[:])

            # ---- all rows' ids in one tile: (p, r, j) = ids[r, 128j+p] ----
            idc_all = sm_pool.tile([128, B_LOC, N_STILE], i32, bufs=1)
            nc.sync.dma_start(
                idc_all[:], ids[:, :].rearrange("r (j p) -> p r j", p=128)
            )
            idc_f_all = sm_pool.tile([128, B_LOC, N_STILE], f32, bufs=1)
            nc.scalar.copy(idc_f_all[:], idc_all[:])

            c3 = [128, N_STILE, 128]
            for r in range(B_LOC):
                # ---- matvec ----
                twraw = sm_pool.tile([128, N_STILE], f32, tag="twraw")
                for j in range(N_STILE):
                    ht = h_pool.tile([128, H], f32, tag="h")
                    nc.sync.dma_start(ht[:], hs[r, 128 * j : 128 * (j + 1), :])
                    prod = sc_pool.tile([128, H], f32, tag="prod")
                    nc.vector.scalar_tensor_tensor(
                        out=prod[:],
                        in0=ht[:],
                        scalar=1.0,
                        in1=w_bc[:],
                        op0=Alu.mult,
                        op1=Alu.mult,
                        accum_out=twraw[:, j : j + 1],
                    )

                # ---- per-column transposed ids on PE ----
                idc_fr = idc_f_all[:, r]
                idT_ps = ps_pool.tile([128, S], f32, tag="idT_ps", bufs=1)
                for j in range(N_STILE):
                    nc.tensor.transpose(
                        out=idT_ps[:, 128 * j : 128 * (j + 1)],
                        in_=idc_fr[:, j : j + 1].to_broadcast([128, 128]),
                        identity=ident[:],
                    )

                # ---- relu(x + b), special-token mask ----
                twrelu = sm_pool.tile([128, N_STILE], f32, tag="twrelu")
                nc.scalar.activation(
                    twrelu[:],
                    twraw[:],
                    mybir.ActivationFunctionType.Relu,
                    bias=float(b_val),
                )
                twm = sm_pool.tile([128, N_STILE], f32, tag="twm")
                nc.vector.scalar_tensor_tensor(
                    out=twm[:],
                    in0=idc_fr[:, :],
                    scalar=4.0,
                    in1=twrelu[:],
                    op0=Alu.is_ge,
                    op1=Alu.mult,
                )

                # ---- eq + per-column duplicate-group sums ----
                eq8 = eq_pool.tile([128, S], f32, tag="eq8")
                nc.vector.tensor_tensor(
                    out=eq8[:].rearrange("p (j q) -> p j q", j=N_STILE),
                    in0=idc_fr[:, :, None].to_broadcast(c3),
                    in1=idT_ps[:].rearrange("p (j q) -> p j q", j=N_STILE),
                    op=Alu.is_equal,
                )
                gsum_ps = ps_pool.tile([128, N_STILE], f32, tag="gsum")
                for j in range(N_STILE):
                    nc.tensor.matmul(
                        out=gsum_ps[:, j : j + 1],
                        lhsT=eq8[:, 128 * j : 128 * (j + 1)],
                        rhs=twm[:, j : j + 1],
                        start=True,
                        stop=True,
                    )
                gsum = sm_pool.tile([128, N_STILE], f32, tag="gsumsb")
                nc.scalar.copy(gsum[:], gsum_ps[:])

                # ---- independent bypass scatters, one table section each ----
                for j in range(N_STILE):
                    nc.gpsimd.indirect_dma_start(
                        out=tables[r][j][:],
                        out_offset=bass.IndirectOffsetOnAxis(
                            ap=idc_all[:, r, j : j + 1], axis=0
                        ),
                        in_=gsum[:, j : j + 1],
                        in_offset=None,
                    )

    nc.compile()
    return nc


def _get_nc(b_val: float):
    key = float(b_val)
    if key not in _compiled:
        _compiled[key] = _build(key)
    return _compiled[key]


def kernel(hidden_state, input_ids, W_sparse, b_sparse):
    from concourse.bass_utils import run_bass_kernel_spmd

    hidden_state = np.ascontiguousarray(np.asarray(hidden_state, dtype=np.float32))
    input_ids = np.ascontiguousarray(np.asarray(input_ids, dtype=np.int32))
    W_sparse = np.ascontiguousarray(np.asarray(W_sparse, dtype=np.float32))
    b_val = float(np.asarray(b_sparse).reshape(-1)[0])

    nc = _get_nc(b_val)

    in_maps = []
    for c in range(N_CORES):
        sl = slice(c * B_LOC, (c + 1) * B_LOC)
        in_maps.append(
            {"hs": hidden_state[sl], "ids": input_ids[sl], "w": W_sparse}
        )

    res = run_bass_kernel_spmd(nc, in_maps, list(range(N_CORES)))

    out = np.empty((B, V), dtype=np.float32)
    for c in range(N_CORES):
        for r in range(B_LOC):
            acc = res.results[c][f"t{r}_0"][:V, 0].copy()
            for j in range(1, N_STILE):
                acc += res.results[c][f"t{r}_{j}"][:V, 0]
            out[c * B_LOC + r] = acc
    return out


# revision 20
# speedup vs baseline: 2.3556x; 1.0659x over previous
"""Trainium2 Bass kernel for BGEM3 sparse-embedding head (segment_reduce).

Computes, for inputs hidden_state [B,S,H], input_ids [B,S], W_sparse [1,H],
b_sparse [1]:
    tw = relu(hidden_state @ W_sparse[0] + b_sparse[0])          # [B,S]
    out = zeros([B,V]); out.at[b, ids].max(tw)  (jax scatter-max, which on
    this stack sums duplicate indices); out[:, 0:4] = 0
Sharding: data-parallel over batch across 8 NeuronCores (4 rows per core).

Per core, per batch row (8 column-tiles of 128 tokens; token s = 128j + p):
  1. matvec: fused mult + add-reduce (DVE scalar_tensor_tensor + accum)
     against a W tile broadcast via PE (ones outer product); relu(x+b) on
     ACT; special ids (< 4) masked to 0.
  2. eq[p,j,q] = (id[128j+p] == id[128j+q]): ids transposed per column on PE
     (broadcast-transpose into PSUM), one batched [128,8,128] DVE pass.
     gsum[:, j] = eq_j @ twm[:, j] (one PE matmul per column): every token
     gets its within-column duplicate-group sum.
  3. Column j of row r scatters (plain writes, 128 offsets = one per
     partition, a hardware limit) into its OWN table section at offset
     j*VPAD. Within a column, duplicate offsets all carry the identical
     group sum, so collisions are benign; across columns the sections are
     disjoint, so no ordering constraints exist anywhere -> the 32 scatters
     stream back-to-back on GpSimd. The host sums the 8 sections per row
     while unsharding (the standard gather for a sum-sharded output).
Output tables rely on the runtime's zero-donated output buffers
(run_bass_via_pjrt donates np.zeros as the ExternalOutput backing).
"""

import numpy as np

B, S, H, V = 32, 1024, 1024, 250002
N_CORES = 8
B_LOC = B // N_CORES          # 4 batch rows per core
VPAD = 128 * 1954             # 250112 >= V, divisible by 128
N_STILE = S // 128            # 8 column-tiles per row

_compiled = {}


def _build(b_val: float):
    import concourse.bass as bass
    import concourse.tile as tile
    from concourse import bacc, mybir
    from concourse.masks import make_identity

    f32 = mybir.dt.float32
    i32 = mybir.dt.int32
    Alu = mybir.AluOpType

    nc = bacc.Bacc("TRN2", target_bir_lowering=False, debug=False)

    hs = nc.dram_tensor("hs", [B_LOC, S, H], f32, kind="ExternalInput")
    ids = nc.dram_tensor("ids", [B_LOC, S], i32, kind="ExternalInput")
    w = nc.dram_tensor("w", [1, H], f32, kind="ExternalInput")
    tables = [
        [
            nc.dram_tensor(f"t{r}_{j}", [VPAD, 1], f32, kind="ExternalOutput")
            for j in range(N_STILE)
        ]
        for r in range(B_LOC)
    ]

    with tile.TileContext(nc) as tc:
        with (
            tc.tile_pool(name="const", bufs=1) as const_pool,
            tc.tile_pool(name="h", bufs=6) as h_pool,
            tc.tile_pool(name="eq", bufs=2) as eq_pool,
            tc.tile_pool(name="sc", bufs=2) as sc_pool,
            tc.tile_pool(name="sm", bufs=2) as sm_pool,
            tc.tile_pool(name="ps", bufs=2, space="PSUM") as ps_pool,
        ):
            # ---- W broadcast to 128 partitions via PE outer product ----
            ones1 = const_pool.tile([1, 128], f32)
            nc.vector.memset(ones1[:], 1.0)
            w_row = const_pool.tile([1, H], f32)
            nc.sync.dma_start(w_row[:], w[0:1, :])
            wb_ps = ps_pool.tile([128, H], f32, tag="wb_ps", bufs=1)
            for half in range(2):
                sl = slice(512 * half, 512 * (half + 1))
                nc.tensor.matmul(
                    out=wb_ps[:, sl],
                    lhsT=ones1[:],
                    rhs=w_row[:, sl],
                    start=True,
                    stop=True,
                )
            w_bc = const_pool.tile([128, H], f32)
            nc.scalar.copy(w_bc[:], wb_ps[:])

            ident = const_pool.tile([128, 128], f32)
            make_identity(nc, ident[:])

            # ---- all rows' ids in one tile: (p, r, j) = ids[r, 128j+p] ----
            idc_all = sm_pool.tile([128, B_LOC, N_STILE], i32, bufs=1)
            idc_f_all = sm_pool.tile([128, B_LOC, N_STILE], f32, bufs=1)
            for r in range(B_LOC):
                nc.sync.dma_start(
                    idc_all[:, r],
                    ids[r, :].rearrange("(j p) -> p j", p=128),
                )
                nc.scalar.copy(idc_f_all[:, r], idc_all[:, r])

            c3 = [128, N_STILE, 128]
            for r in range(B_LOC):
                # ---- matvec ----
                twraw = sm_pool.tile([128, N_STILE], f32, tag="twraw")
                for j in range(N_STILE):
                    ht = h_pool.tile([128, H], f32, tag="h")
                    nc.sync.dma_start(ht[:], hs[r, 128 * j : 128 * (j + 1), :])
                    prod = sc_pool.tile([128, H], f32, tag="prod")
                    nc.vector.scalar_tensor_tensor(
                        out=prod[:],
                        in0=ht[:],
                        scalar=1.0,
                        in1=w_bc[:],
                        op0=Alu.mult,
                        op1=Alu.mult,
                        accum_out=twraw[:, j : j + 1],
                    )

                # ---- per-column transposed ids on PE ----
                idc_fr = idc_f_all[:, r]
                idT_ps = ps_pool.tile([128, S], f32, tag="idT_ps", bufs=1)
                for j in range(N_STILE):
                    nc.tensor.transpose(
                        out=idT_ps[:, 128 * j : 128 * (j + 1)],
                        in_=idc_fr[:, j : j + 1].to_broadcast([128, 128]),
                        identity=ident[:],
                    )

                # ---- relu(x + b), special-token mask ----
                twrelu = sm_pool.tile([128, N_STILE], f32, tag="twrelu")
                nc.scalar.activation(
                    twrelu[:],
                    twraw[:],
                    mybir.ActivationFunctionType.Relu,
                    bias=float(b_val),
                )
                twm = sm_pool.tile([128, N_STILE], f32, tag="twm")
                nc.vector.scalar_tensor_tensor(
                    out=twm[:],
                    in0=idc_fr[:, :],
                    scalar=4.0,
                    in1=twrelu[:],
                    op0=Alu.is_ge,
                    op1=Alu.mult,
                )

                # ---- eq + per-column duplicate-group sums ----
                eq8 = eq_pool.tile([128, S], f32, tag="eq8")
                nc.vector.tensor_tensor(
                    out=eq8[:].rearrange("p (j q) -> p j q", j=N_STILE),
                    in0=idc_fr[:, :, None].to_broadcast(c3),
                    in1=idT_ps[:].rearrange("p (j q) -> p j q", j=N_STILE),
                    op=Alu.is_equal,
                )
                gsum_ps = ps_pool.tile([128, N_STILE], f32, tag="gsum")
                for j in range(N_STILE):
                    nc.tensor.matmul(
                        out=gsum_ps[:, j : j + 1],
                        lhsT=eq8[:, 128 * j : 128 * (j + 1)],
                        rhs=twm[:, j : j + 1],
                        start=True,
                        stop=True,
                    )
                gsum = sm_pool.tile([128, N_STILE], f32, tag="gsumsb")

                # ---- independent bypass scatters, one table each ----
                for j in range(N_STILE):
                    nc.scalar.copy(gsum[:, j : j + 1], gsum_ps[:, j : j + 1])
                    nc.gpsimd.indirect_dma_start(
                        out=tables[r][j][:],
                        out_offset=bass.IndirectOffsetOnAxis(
                            ap=idc_all[:, r, j : j + 1], axis=0
                        ),
                        in_=gsum[:, j : j + 1],
                        in_offset=None,
                    )

    nc.compile()
    return nc


def _get_nc(b_val: float):
    key = float(b_val)
    if key not in _compiled:
        _compiled[key] = _build(key)
    return _compiled[key]


def kernel(hidden_state, input_ids, W_sparse, b_sparse):
    from concourse.bass_utils import run_bass_kernel_spmd

    hidden_state = np.ascontiguousarray(np.asarray(hidden_state, dtype=np.float32))
    input_ids = np.ascontiguousarray(np.asarray(input_ids, dtype=np.int32))
    W_sparse = np.ascontiguousarray(np.asarray(W_sparse, dtype=np.float32))
    b_val = float(np.asarray(b_sparse).reshape(-1)[0])

    nc = _get_nc(b_val)

    in_maps = []
    for c in range(N_CORES):
        sl = slice(c * B_LOC, (c + 1) * B_LOC)
        in_maps.append(
            {"hs": hidden_state[sl], "ids": input_ids[sl], "w": W_sparse}
        )

    res = run_bass_kernel_spmd(nc, in_maps, list(range(N_CORES)))

    out = np.empty((B, V), dtype=np.float32)
    for c in range(N_CORES):
        for r in range(B_LOC):
            acc = res.results[c][f"t{r}_0"][:V, 0].copy()
            for j in range(1, N_STILE):
                acc += res.results[c][f"t{r}_{j}"][:V, 0]
            out[c * B_LOC + r] = acc
    return out


# revision 21
# speedup vs baseline: 2.4920x; 1.0579x over previous
"""Trainium2 Bass kernel for BGEM3 sparse-embedding head (segment_reduce).

Computes, for inputs hidden_state [B,S,H], input_ids [B,S], W_sparse [1,H],
b_sparse [1]:
    tw = relu(hidden_state @ W_sparse[0] + b_sparse[0])          # [B,S]
    out = zeros([B,V]); out.at[b, ids].max(tw)  (jax scatter-max, which on
    this stack sums duplicate indices); out[:, 0:4] = 0
Sharding: data-parallel over batch across 8 NeuronCores (4 rows per core).

Per core, per batch row (8 column-tiles of 128 tokens; token s = 128j + p):
  1. matvec: fused mult + add-reduce (DVE scalar_tensor_tensor + accum)
     against a W tile broadcast via PE (ones outer product); relu(x+b) on
     ACT; special ids (< 4) masked to 0.
  2. eq[p,j,q] = (id[128j+p] == id[128j+q]): ids transposed per column on PE
     (broadcast-transpose into PSUM), one batched [128,8,128] DVE pass.
     gsum[:, j] = eq_j @ twm[:, j] (one PE matmul per column): every token
     gets its within-column duplicate-group sum.
  3. Column j of row r scatters (plain writes, 128 offsets = one per
     partition, a hardware limit) into its OWN table section at offset
     j*VPAD. Within a column, duplicate offsets all carry the identical
     group sum, so collisions are benign; across columns the sections are
     disjoint, so no ordering constraints exist anywhere -> the 32 scatters
     stream back-to-back on GpSimd. The host sums the 8 sections per row
     while unsharding (the standard gather for a sum-sharded output).
Output tables rely on the runtime's zero-donated output buffers
(run_bass_via_pjrt donates np.zeros as the ExternalOutput backing).
"""

import numpy as np

B, S, H, V = 32, 1024, 1024, 250002
N_CORES = 8
B_LOC = B // N_CORES          # 4 batch rows per core
VPAD = 128 * 1954             # 250112 >= V, divisible by 128
N_STILE = S // 128            # 8 column-tiles per row

_compiled = {}


def _build(b_val: float):
    import concourse.bass as bass
    import concourse.tile as tile
    from concourse import bacc, mybir
    from concourse.masks import make_identity

    f32 = mybir.dt.float32
    i32 = mybir.dt.int32
    Alu = mybir.AluOpType

    nc = bacc.Bacc("TRN2", target_bir_lowering=False, debug=False)

    hs = nc.dram_tensor("hs", [B_LOC, S, H], f32, kind="ExternalInput")
    ids = nc.dram_tensor("ids", [B_LOC, S], i32, kind="ExternalInput")
    w = nc.dram_tensor("w", [1, H], f32, kind="ExternalInput")
    tables = [
        [
            nc.dram_tensor(f"t{r}_{j}", [VPAD, 1], f32, kind="ExternalOutput")
            for j in range(N_STILE)
        ]
        for r in range(B_LOC)
    ]

    with tile.TileContext(nc) as tc:
        with (
            tc.tile_pool(name="const", bufs=1) as const_pool,
            tc.tile_pool(name="h", bufs=6) as h_pool,
            tc.tile_pool(name="eq", bufs=2) as eq_pool,
            tc.tile_pool(name="sc", bufs=2) as sc_pool,
            tc.tile_pool(name="sm", bufs=2) as sm_pool,
            tc.tile_pool(name="ps", bufs=2, space="PSUM") as ps_pool,
        ):
            # ---- W broadcast to 128 partitions via PE outer product ----
            ones1 = const_pool.tile([1, 128], f32)
            nc.vector.memset(ones1[:], 1.0)
            w_row = const_pool.tile([1, H], f32)
            nc.sync.dma_start(w_row[:], w[0:1, :])
            wb_ps = ps_pool.tile([128, H], f32, tag="wb_ps", bufs=1)
            for half in range(2):
                sl = slice(512 * half, 512 * (half + 1))
                nc.tensor.matmul(
                    out=wb_ps[:, sl],
                    lhsT=ones1[:],
                    rhs=w_row[:, sl],
                    start=True,
                    stop=True,
                )
            w_bc = const_pool.tile([128, H], f32)
            nc.scalar.copy(w_bc[:], wb_ps[:])

            ident = const_pool.tile([128, 128], f32)
            make_identity(nc, ident[:])

            # ---- all rows' ids in one tile: (p, r, j) = ids[r, 128j+p] ----
            idc_all = sm_pool.tile([128, B_LOC, N_STILE], i32, bufs=1)
            idc_f_all = sm_pool.tile([128, B_LOC, N_STILE], f32, bufs=1)
            for r in range(B_LOC):
                nc.sync.dma_start(
                    idc_all[:, r],
                    ids[r, :].rearrange("(j p) -> p j", p=128),
                )
                nc.scalar.copy(idc_f_all[:, r], idc_all[:, r])

            c3 = [128, N_STILE, 128]
            HALF = N_STILE // 2
            for r in range(B_LOC):
                idc_fr = idc_f_all[:, r]

                # ---- id-only work first: transposed ids (PE) + eq ----
                idT_ps = ps_pool.tile([128, S], f32, tag="idT_ps", bufs=1)
                for j in range(N_STILE):
                    nc.tensor.transpose(
                        out=idT_ps[:, 128 * j : 128 * (j + 1)],
                        in_=idc_fr[:, j : j + 1].to_broadcast([128, 128]),
                        identity=ident[:],
                    )
                eq8 = eq_pool.tile([128, S], f32, tag="eq8")
                nc.vector.tensor_tensor(
                    out=eq8[:].rearrange("p (j q) -> p j q", j=N_STILE),
                    in0=idc_fr[:, :, None].to_broadcast(c3),
                    in1=idT_ps[:].rearrange("p (j q) -> p j q", j=N_STILE),
                    op=Alu.is_equal,
                )

                # ---- matvec + per-half relu/mask/gsum/scatter chains ----
                twraw = sm_pool.tile([128, N_STILE], f32, tag="twraw")
                twrelu = sm_pool.tile([128, N_STILE], f32, tag="twrelu")
                twm = sm_pool.tile([128, N_STILE], f32, tag="twm")
                gsum_ps = ps_pool.tile([128, N_STILE], f32, tag="gsum")
                gsum = sm_pool.tile([128, N_STILE], f32, tag="gsumsb")
                for j in range(N_STILE):
                    ht = h_pool.tile([128, H], f32, tag="h")
                    nc.sync.dma_start(ht[:], hs[r, 128 * j : 128 * (j + 1), :])
                    prod = sc_pool.tile([128, H], f32, tag="prod")
                    nc.vector.scalar_tensor_tensor(
                        out=prod[:],
                        in0=ht[:],
                        scalar=1.0,
                        in1=w_bc[:],
                        op0=Alu.mult,
                        op1=Alu.mult,
                        accum_out=twraw[:, j : j + 1],
                    )
                    if j % HALF != HALF - 1:
                        continue
                    # half-row [j-HALF+1 .. j] is complete: finish it
                    h0 = j - HALF + 1
                    sl = slice(h0, j + 1)
                    nc.scalar.activation(
                        twrelu[:, sl],
                        twraw[:, sl],
                        mybir.ActivationFunctionType.Relu,
                        bias=float(b_val),
                    )
                    nc.vector.scalar_tensor_tensor(
                        out=twm[:, sl],
                        in0=idc_fr[:, sl],
                        scalar=4.0,
                        in1=twrelu[:, sl],
                        op0=Alu.is_ge,
                        op1=Alu.mult,
                    )
                    for jj in range(h0, j + 1):
                        nc.tensor.matmul(
                            out=gsum_ps[:, jj : jj + 1],
                            lhsT=eq8[:, 128 * jj : 128 * (jj + 1)],
                            rhs=twm[:, jj : jj + 1],
                            start=True,
                            stop=True,
                        )
                        nc.scalar.copy(
                            gsum[:, jj : jj + 1], gsum_ps[:, jj : jj + 1]
                        )
                        nc.gpsimd.indirect_dma_start(
                            out=tables[r][jj][:],
                            out_offset=bass.IndirectOffsetOnAxis(
                                ap=idc_all[:, r, jj : jj + 1], axis=0
                            ),
                            in_=gsum[:, jj : jj + 1],
                            in_offset=None,
                        )

    nc.compile()
    return nc


def _get_nc(b_val: float):
    key = float(b_val)
    if key not in _compiled:
        _compiled[key] = _build(key)
    return _compiled[key]


def kernel(hidden_state, input_ids, W_sparse, b_sparse):
    from concourse.bass_utils import run_bass_kernel_spmd

    hidden_state = np.ascontiguousarray(np.asarray(hidden_state, dtype=np.float32))
    input_ids = np.ascontiguousarray(np.asarray(input_ids, dtype=np.int32))
    W_sparse = np.ascontiguousarray(np.asarray(W_sparse, dtype=np.float32))
    b_val = float(np.asarray(b_sparse).reshape(-1)[0])

    nc = _get_nc(b_val)

    in_maps = []
    for c in range(N_CORES):
        sl = slice(c * B_LOC, (c + 1) * B_LOC)
        in_maps.append(
            {"hs": hidden_state[sl], "ids": input_ids[sl], "w": W_sparse}
        )

    res = run_bass_kernel_spmd(nc, in_maps, list(range(N_CORES)))

    out = np.empty((B, V), dtype=np.float32)
    for c in range(N_CORES):
        for r in range(B_LOC):
            acc = res.results[c][f"t{r}_0"][:V, 0].copy()
            for j in range(1, N_STILE):
                acc += res.results[c][f"t{r}_{j}"][:V, 0]
            out[c * B_LOC + r] = acc
    return out


# revision 23
# speedup vs baseline: 2.6199x; 1.0513x over previous
"""Trainium2 Bass kernel for BGEM3 sparse-embedding head (segment_reduce).

Computes, for inputs hidden_state [B,S,H], input_ids [B,S], W_sparse [1,H],
b_sparse [1]:
    tw = relu(hidden_state @ W_sparse[0] + b_sparse[0])          # [B,S]
    out = zeros([B,V]); out.at[b, ids].max(tw)  (jax scatter-max, which on
    this stack sums duplicate indices); out[:, 0:4] = 0
Sharding: data-parallel over batch across 8 NeuronCores (4 rows per core).

Per core, per batch row (8 column-tiles of 128 tokens; token s = 128j + p):
  1. matvec: fused mult + add-reduce (DVE scalar_tensor_tensor + accum)
     against a W tile broadcast via PE (ones outer product); relu(x+b) on
     ACT; special ids (< 4) masked to 0.
  2. eq[p,j,q] = (id[128j+p] == id[128j+q]): ids transposed per column on PE
     (broadcast-transpose into PSUM), one batched [128,8,128] DVE pass.
     gsum[:, j] = eq_j @ twm[:, j] (one PE matmul per column): every token
     gets its within-column duplicate-group sum.
  3. Column j of row r scatters (plain writes, 128 offsets = one per
     partition, a hardware limit) into its OWN table section at offset
     j*VPAD. Within a column, duplicate offsets all carry the identical
     group sum, so collisions are benign; across columns the sections are
     disjoint, so no ordering constraints exist anywhere -> the 32 scatters
     stream back-to-back on GpSimd. The host sums the 8 sections per row
     while unsharding (the standard gather for a sum-sharded output).
Output tables rely on the runtime's zero-donated output buffers
(run_bass_via_pjrt donates np.zeros as the ExternalOutput backing).
"""

import numpy as np

B, S, H, V = 32, 1024, 1024, 250002
N_CORES = 8
B_LOC = B // N_CORES          # 4 batch rows per core
VPAD = 128 * 1954             # 250112 >= V, divisible by 128
N_STILE = S // 128            # 8 column-tiles per row

_compiled = {}


def _build(b_val: float):
    import concourse.bass as bass
    import concourse.tile as tile
    from concourse import bacc, mybir
    from concourse.masks import make_identity

    f32 = mybir.dt.float32
    i32 = mybir.dt.int32
    Alu = mybir.AluOpType

    nc = bacc.Bacc("TRN2", target_bir_lowering=False, debug=False)

    hs = nc.dram_tensor("hs", [B_LOC, S, H], f32, kind="ExternalInput")
    ids = nc.dram_tensor("ids", [B_LOC, S], i32, kind="ExternalInput")
    w = nc.dram_tensor("w", [1, H], f32, kind="ExternalInput")
    tables = [
        [
            nc.dram_tensor(f"t{r}_{j}", [VPAD, 1], f32, kind="ExternalOutput")
            for j in range(N_STILE)
        ]
        for r in range(B_LOC)
    ]

    with tile.TileContext(nc) as tc:
        with (
            tc.tile_pool(name="const", bufs=1) as const_pool,
            tc.tile_pool(name="h", bufs=6) as h_pool,
            tc.tile_pool(name="eq", bufs=2) as eq_pool,
            tc.tile_pool(name="sc", bufs=2) as sc_pool,
            tc.tile_pool(name="sm", bufs=2) as sm_pool,
            tc.tile_pool(name="ps", bufs=2, space="PSUM") as ps_pool,
        ):
            # ---- W broadcast to 128 partitions via PE outer product ----
            ones1 = const_pool.tile([1, 128], f32)
            nc.vector.memset(ones1[:], 1.0)
            w_row = const_pool.tile([1, H], f32)
            nc.sync.dma_start(w_row[:], w[0:1, :])
            wb_ps = ps_pool.tile([128, H], f32, tag="wb_ps", bufs=1)
            for half in range(2):
                sl = slice(512 * half, 512 * (half + 1))
                nc.tensor.matmul(
                    out=wb_ps[:, sl],
                    lhsT=ones1[:],
                    rhs=w_row[:, sl],
                    start=True,
                    stop=True,
                )
            w_bc = const_pool.tile([128, H], f32)
            nc.scalar.copy(w_bc[:], wb_ps[:])

            ident = const_pool.tile([128, 128], f32)
            make_identity(nc, ident[:])

            # ---- all rows' ids in one tile: (p, r, j) = ids[r, 128j+p] ----
            idc_all = sm_pool.tile([128, B_LOC, N_STILE], i32, bufs=1)
            idc_f_all = sm_pool.tile([128, B_LOC, N_STILE], f32, bufs=1)
            for r in range(B_LOC):
                nc.sync.dma_start(
                    idc_all[:, r].rearrange("p (blk k) -> p blk k", k=2),
                    ids[r, :].rearrange("(blk p k) -> p blk k", p=128, k=2),
                )
                nc.scalar.copy(idc_f_all[:, r], idc_all[:, r])

            c3 = [128, N_STILE, 128]
            HALF = N_STILE // 2
            for r in range(B_LOC):
                idc_fr = idc_f_all[:, r]

                # ---- id-only work first: transposed ids (PE) + eq ----
                idT_ps = ps_pool.tile([128, S], f32, tag="idT_ps", bufs=1)
                for j in range(N_STILE):
                    nc.tensor.transpose(
                        out=idT_ps[:, 128 * j : 128 * (j + 1)],
                        in_=idc_fr[:, j : j + 1].to_broadcast([128, 128]),
                        identity=ident[:],
                    )
                eq8 = eq_pool.tile([128, S], f32, tag="eq8")
                nc.vector.tensor_tensor(
                    out=eq8[:].rearrange("p (j q) -> p j q", j=N_STILE),
                    in0=idc_fr[:, :, None].to_broadcast(c3),
                    in1=idT_ps[:].rearrange("p (j q) -> p j q", j=N_STILE),
                    op=Alu.is_equal,
                )

                # ---- matvec + per-half relu/mask/gsum/scatter chains ----
                twraw = sm_pool.tile([128, N_STILE], f32, tag="twraw")
                twrelu = sm_pool.tile([128, N_STILE], f32, tag="twrelu")
                twm = sm_pool.tile([128, N_STILE], f32, tag="twm")
                gsum_ps = ps_pool.tile([128, N_STILE], f32, tag="gsum")
                gsum = sm_pool.tile([128, N_STILE], f32, tag="gsumsb")
                for blk in range(N_STILE // 2):
                    ht = h_pool.tile([128, 2, H], f32, tag="h")
                    nc.sync.dma_start(
                        ht[:],
                        hs[r, 256 * blk : 256 * (blk + 1), :].rearrange(
                            "(p k) h -> p k h", p=128
                        ),
                    )
                    for k in range(2):
                        j = 2 * blk + k
                        prod = sc_pool.tile([128, H], f32, tag="prod")
                        nc.vector.scalar_tensor_tensor(
                            out=prod[:],
                            in0=ht[:, k],
                            scalar=1.0,
                            in1=w_bc[:],
                            op0=Alu.mult,
                            op1=Alu.mult,
                            accum_out=twraw[:, j : j + 1],
                        )
                    j = 2 * blk + 1
                    if j % HALF != HALF - 1:
                        continue
                    # half-row [j-HALF+1 .. j] is complete: finish it
                    h0 = j - HALF + 1
                    sl = slice(h0, j + 1)
                    nc.scalar.activation(
                        twrelu[:, sl],
                        twraw[:, sl],
                        mybir.ActivationFunctionType.Relu,
                        bias=float(b_val),
                    )
                    nc.vector.scalar_tensor_tensor(
                        out=twm[:, sl],
                        in0=idc_fr[:, sl],
                        scalar=4.0,
                        in1=twrelu[:, sl],
                        op0=Alu.is_ge,
                        op1=Alu.mult,
                    )
                    for jj in range(h0, j + 1):
                        nc.tensor.matmul(
                            out=gsum_ps[:, jj : jj + 1],
                            lhsT=eq8[:, 128 * jj : 128 * (jj + 1)],
                            rhs=twm[:, jj : jj + 1],
                            start=True,
                            stop=True,
                        )
                        nc.scalar.copy(
                            gsum[:, jj : jj + 1], gsum_ps[:, jj : jj + 1]
                        )
                        nc.gpsimd.indirect_dma_start(
                            out=tables[r][jj][:],
                            out_offset=bass.IndirectOffsetOnAxis(
                                ap=idc_all[:, r, jj : jj + 1], axis=0
                            ),
                            in_=gsum[:, jj : jj + 1],
                            in_offset=None,
                        )

    nc.compile()
    return nc


def _get_nc(b_val: float):
    key = float(b_val)
    if key not in _compiled:
        _compiled[key] = _build(key)
    return _compiled[key]


def kernel(hidden_state, input_ids, W_sparse, b_sparse):
    from concourse.bass_utils import run_bass_kernel_spmd

    hidden_state = np.ascontiguousarray(np.asarray(hidden_state, dtype=np.float32))
    input_ids = np.ascontiguousarray(np.asarray(input_ids, dtype=np.int32))
    W_sparse = np.ascontiguousarray(np.asarray(W_sparse, dtype=np.float32))
    b_val = float(np.asarray(b_sparse).reshape(-1)[0])

    nc = _get_nc(b_val)

    in_maps = []
    for c in range(N_CORES):
        sl = slice(c * B_LOC, (c + 1) * B_LOC)
        in_maps.append(
            {"hs": hidden_state[sl], "ids": input_ids[sl], "w": W_sparse}
        )

    res = run_bass_kernel_spmd(nc, in_maps, list(range(N_CORES)))

    out = np.empty((B, V), dtype=np.float32)
    for c in range(N_CORES):
        for r in range(B_LOC):
            acc = res.results[c][f"t{r}_0"][:V, 0].copy()
            for j in range(1, N_STILE):
                acc += res.results[c][f"t{r}_{j}"][:V, 0]
            out[c * B_LOC + r] = acc
    return out
